# revision 11
# baseline (speedup 1.0000x reference)
"""Trainium2 Bass kernel: batched bond-angle cosines (gather + vector math).

Problem: geometry (n_atoms, 3, batch) f32, angle triplets (n_angles, 3) int32.
Output: cos(angle) per (triplet, frame) = (n_angles, batch) f32.

Architecture (v3):
- Shard angles across 8 cores (8192 each, padded to 66 groups x 126).
- Geometry viewed as a (n_atoms*3, batch) row table; row 3a+c = coords of
  atom a, coordinate c. Partition layout packs 42 angles x 3 coords into
  126 of 128 partitions, so the per-coordinate values of an angle's three
  endpoint vectors live in adjacent partitions.
- Gather via gpsimd.dma_gather (batched DGE gather, int16 indices), one call
  per 2 groups (2304 rows of batch*4 bytes).
- Per 42-angle subtile: d1 = a-b (DVE), d2 = c-b (GpSimd), m = d1*d2 (DVE),
  q1 = d1^2, q2 = d2^2 (ACT). The 3-term coordinate sums (dot, |v1|^2,
  |v2|^2) are done by the PE array as float32r matmuls against a constant
  0/1 selection matrix, accumulating 3 subtiles into one 126-row PSUM tile.
- Tail per group: t = n1*n2 (DVE), s = sqrt(t) (ACT), r ~= 1/s (custom DVE
  reciprocal_approx_fast; r(0) = NaN preserving the reference's 0/0 NaN
  semantics), res = dot*r (DVE). Contiguous 126-row DMA to the output.
"""

import numpy as np

import concourse.tile as tile
from concourse import bacc, bass, mybir
from concourse.bass_utils import run_bass_kernel_spmd

P = 128
APT = 42          # angles per subtile (3 coords each -> 126 partitions)
GROUP = 3 * APT   # angles per PSUM group = 126
GPC = 2           # groups per dma_gather call

N_ATOMS = 2048
N_ANGLES = 65536
BATCH = 512
N_CORES = 8
PER_CORE = N_ANGLES // N_CORES          # 8192
N_GROUPS = -(-PER_CORE // GROUP)        # 66
PAD_ANGLES = N_GROUPS * GROUP           # 8316
N_CALLS = N_GROUPS // GPC               # 33

_NC_CACHE = {}


def build_nc(n_atoms=N_ATOMS, n_groups=N_GROUPS, batch=BATCH):
    assert n_groups % GPC == 0
    n_calls = n_groups // GPC
    B = batch
    rows_per_window = GPC * 9 * P  # 2 groups x (3 subtiles x 3 roles) x 128
    rows_per_call = rows_per_window // 3  # dma_gather caps at 1024 idxs/call
    f32 = mybir.dt.float32
    f32r = mybir.dt.float32r
    i16 = mybir.dt.int16

    nc = bacc.Bacc(debug=False)

    geom = nc.declare_dram_parameter("geom", [3 * n_atoms, B], f32, isOutput=False)
    idxs = nc.declare_dram_parameter(
        "idxs", [P, n_calls * rows_per_window // 16], i16, isOutput=False
    )
    wmat = nc.declare_dram_parameter("wmat", [P, 3 * P], f32, isOutput=False)
    out = nc.declare_dram_parameter("out", [n_groups * GROUP, B], f32, isOutput=True)

    idx_cols = rows_per_call // 16   # i16 columns per dma_gather call
    win_cols = rows_per_window // 16

    with tile.TileContext(nc) as tc:
        with (
            tc.tile_pool(name="const", bufs=1) as constp,
            tc.tile_pool(name="gath", bufs=2) as gath,
            tc.tile_pool(name="work", bufs=3) as work,
            tc.tile_pool(name="tailp", bufs=2) as tailp,
            tc.tile_pool(name="psum", bufs=2, space="PSUM") as psum,
        ):
            idx_sb = constp.tile([P, n_calls * win_cols], i16)
            nc.sync.dma_start(out=idx_sb[:, :], in_=idxs[:, :])
            w_raw = constp.tile([P, 3 * P], f32)
            nc.sync.dma_start(out=w_raw[:, :], in_=wmat[:, :])
            w_sb = constp.tile([P, 3 * P], f32r)
            nc.vector.tensor_copy(w_sb[:, :], w_raw[:, :])

            slots_per_call = rows_per_call // P
            for call in range(n_calls):
                gbuf = gath.tile([P, GPC * 9, B], f32, tag="gbuf")
                for sub in range(3):
                    col0 = call * win_cols + sub * idx_cols
                    nc.gpsimd.dma_gather(
                        out_ap=gbuf[
                            :, sub * slots_per_call : (sub + 1) * slots_per_call, :
                        ],
                        in_ap=geom[:, :],
                        idxs_ap=idx_sb[:, col0 : col0 + idx_cols],
                        num_idxs=rows_per_call,
                        num_idxs_reg=rows_per_call,
                        elem_size=B,
                    )
                for gg in range(GPC):
                    g = call * GPC + gg
                    dotP = psum.tile([P, B], f32, tag="dotP")
                    n1P = psum.tile([P, B], f32, tag="n1P")
                    n2P = psum.tile([P, B], f32, tag="n2P")
                    for j in range(3):
                        ga = gbuf[:, 9 * gg + 3 * j + 0, :]
                        gb = gbuf[:, 9 * gg + 3 * j + 1, :]
                        gc = gbuf[:, 9 * gg + 3 * j + 2, :]
                        d1 = work.tile([P, B], f32, tag="d1")
                        d2 = work.tile([P, B], f32, tag="d2")
                        m = work.tile([P, B], f32r, tag="m")
                        q1 = work.tile([P, B], f32r, tag="q1")
                        q2 = work.tile([P, B], f32r, tag="q2")
                        nc.vector.tensor_sub(d1[:, :], ga, gb)
                        nc.gpsimd.tensor_sub(d2[:, :], gc, gb)
                        nc.vector.tensor_mul(m[:, :], d1[:, :], d2[:, :])
                        nc.scalar.square(q1[:, :], d1[:, :])
                        nc.scalar.square(q2[:, :], d2[:, :])
                        wj = w_sb[:, j * P : (j + 1) * P]
                        st = dict(start=(j == 0), stop=(j == 2))
                        nc.tensor.matmul(
                            out=dotP[:, :], lhsT=wj, rhs=m[:, :], **st
                        )
                        nc.tensor.matmul(
                            out=n1P[:, :], lhsT=wj, rhs=q1[:, :], **st
                        )
                        nc.tensor.matmul(
                            out=n2P[:, :], lhsT=wj, rhs=q2[:, :], **st
                        )

                    s1 = tailp.tile([P, B], f32, tag="s1")
                    s2 = tailp.tile([P, B], f32, tag="s2")
                    s = tailp.tile([P, B], f32, tag="s")
                    r = tailp.tile([P, B], f32, tag="r")
                    res = tailp.tile([P, B], f32, tag="res")
                    nc.scalar.sqrt(s1[:, :], n1P[:, :])
                    nc.scalar.sqrt(s2[:, :], n2P[:, :])
                    nc.vector.tensor_mul(s[:, :], s1[:, :], s2[:, :])
                    nc.vector.reciprocal_approx_fast(r[:, :], s[:, :])
                    nc.vector.tensor_mul(res[:, :], dotP[:, :], r[:, :])
                    nc.sync.dma_start(
                        out=out[g * GROUP : (g + 1) * GROUP, :], in_=res[:GROUP, :]
                    )

    nc.compile()
    return nc


def _build_wmat():
    W = np.zeros((P, 3, P), np.float32)
    a = np.arange(APT)
    for j in range(3):
        for c in range(3):
            W[3 * a + c, j, APT * j + a] = 1.0
    return np.ascontiguousarray(W.reshape(P, 3 * P))


def _prep_core_idxs(angles, core, n_groups):
    """int16 wrapped index array (P, n_calls*idx_cols) for this core."""
    pad_angles = n_groups * GROUP
    ang = angles[core * PER_CORE : core * PER_CORE + pad_angles]
    if ang.shape[0] < pad_angles:
        pad = np.zeros((pad_angles - ang.shape[0], 3), ang.dtype)
        ang = np.concatenate([ang, pad], axis=0)
    A = ang.reshape(n_groups, 3, APT, 3)  # [g, j, a_local, role]
    p = np.arange(P)
    al, c = p // 3, p % 3
    valid = p < GROUP
    idx = np.zeros((n_groups, 3, 3, P), np.int16)
    # [g, j, role, p] = 3*atom + coord
    sel = (3 * A[:, :, al[valid], :] + c[valid][None, None, :, None]).transpose(
        0, 1, 3, 2
    )
    idx[:, :, :, valid] = sel.astype(np.int16)
    n_subcalls = (n_groups // GPC) * 3
    flat = idx.reshape(n_subcalls, GPC * 9 * P // 3)  # rows per dma_gather call
    # wrap: index i lives at partition i%16, free column i//16; replicate x8
    wr = flat.reshape(n_subcalls, -1, 16)  # [call, s, i]
    wrapped = np.tile(wr.transpose(0, 2, 1), (1, 8, 1))  # [call, 128, S]
    return np.ascontiguousarray(
        wrapped.transpose(1, 0, 2).reshape(P, -1)
    )


def kernel(input, angles, _trace=False, _trace_kwargs=None):
    input = np.ascontiguousarray(np.asarray(input, dtype=np.float32))
    angles = np.ascontiguousarray(np.asarray(angles, dtype=np.int32))
    n_atoms = input.shape[0]
    batch = input.shape[2]
    geom = input.reshape(3 * n_atoms, batch)

    key = (n_atoms, N_GROUPS, batch)
    if key not in _NC_CACHE:
        _NC_CACHE[key] = build_nc(*key)
    nc = _NC_CACHE[key]

    wm = _build_wmat()
    in_maps = [
        {"geom": geom, "idxs": _prep_core_idxs(angles, c, N_GROUPS), "wmat": wm}
        for c in range(N_CORES)
    ]
    kw = {}
    if _trace:
        kw["trace"] = True
        kw.update(_trace_kwargs or {})
    res = run_bass_kernel_spmd(nc, in_maps, core_ids=list(range(N_CORES)), **kw)
    outs = [res.results[c]["out"][:PER_CORE] for c in range(N_CORES)]
    full = np.concatenate(outs, axis=0)
    if _trace:
        return full, res
    return full


# revision 12
# speedup vs baseline: 1.1000x; 1.1000x over previous
"""Trainium2 Bass kernel: batched bond-angle cosines (gather + vector math).

Problem: geometry (n_atoms, 3, batch) f32, angle triplets (n_angles, 3) int32.
Output: cos(angle) per (triplet, frame) = (n_angles, batch) f32.

Architecture (v3):
- Shard angles across 8 cores (8192 each, padded to 66 groups x 126).
- Geometry viewed as a (n_atoms*3, batch) row table; row 3a+c = coords of
  atom a, coordinate c. Partition layout packs 42 angles x 3 coords into
  126 of 128 partitions, so the per-coordinate values of an angle's three
  endpoint vectors live in adjacent partitions.
- Gather via gpsimd.dma_gather (batched DGE gather, int16 indices), one call
  per 2 groups (2304 rows of batch*4 bytes).
- Per 42-angle subtile: d1 = a-b (DVE), d2 = c-b (GpSimd), m = d1*d2 (DVE),
  q1 = d1^2, q2 = d2^2 (ACT). The 3-term coordinate sums (dot, |v1|^2,
  |v2|^2) are done by the PE array as float32r matmuls against a constant
  0/1 selection matrix, accumulating 3 subtiles into one 126-row PSUM tile.
- Tail per group: t = n1*n2 (DVE), s = sqrt(t) (ACT), r ~= 1/s (custom DVE
  reciprocal_approx_fast; r(0) = NaN preserving the reference's 0/0 NaN
  semantics), res = dot*r (DVE). Contiguous 126-row DMA to the output.
"""

import numpy as np

import concourse.tile as tile
from concourse import bacc, bass, mybir
from concourse.bass_utils import run_bass_kernel_spmd

P = 128
APT = 42          # angles per subtile (3 coords each -> 126 partitions)
GROUP = 3 * APT   # angles per PSUM group = 126
GPC = 2           # groups per dma_gather call

N_ATOMS = 2048
N_ANGLES = 65536
BATCH = 512
N_CORES = 8
PER_CORE = N_ANGLES // N_CORES          # 8192
N_GROUPS = -(-PER_CORE // GROUP)        # 66
PAD_ANGLES = N_GROUPS * GROUP           # 8316
N_CALLS = N_GROUPS // GPC               # 33

_NC_CACHE = {}


def build_nc(n_atoms=N_ATOMS, n_groups=N_GROUPS, batch=BATCH):
    assert n_groups % GPC == 0
    n_calls = n_groups // GPC
    B = batch
    rows_per_window = GPC * 9 * P  # 2 groups x (3 subtiles x 3 roles) x 128
    rows_per_call = rows_per_window // 3  # dma_gather caps at 1024 idxs/call
    f32 = mybir.dt.float32
    f32r = mybir.dt.float32r
    i16 = mybir.dt.int16

    nc = bacc.Bacc(debug=False)

    geom = nc.declare_dram_parameter("geom", [3 * n_atoms, B], f32, isOutput=False)
    idxs = nc.declare_dram_parameter(
        "idxs", [P, n_calls * rows_per_window // 16], i16, isOutput=False
    )
    wmat = nc.declare_dram_parameter("wmat", [P, 3 * P], f32, isOutput=False)
    out = nc.declare_dram_parameter("out", [n_groups * GROUP, B], f32, isOutput=True)

    idx_cols = rows_per_call // 16   # i16 columns per dma_gather call
    win_cols = rows_per_window // 16

    with tile.TileContext(nc) as tc:
        with (
            tc.tile_pool(name="const", bufs=1) as constp,
            tc.tile_pool(name="gath", bufs=3) as gath,
            tc.tile_pool(name="work", bufs=3) as work,
            tc.tile_pool(name="tailp", bufs=2) as tailp,
            tc.tile_pool(name="psum", bufs=2, space="PSUM") as psum,
        ):
            idx_sb = constp.tile([P, n_calls * win_cols], i16)
            nc.sync.dma_start(out=idx_sb[:, :], in_=idxs[:, :])
            w_raw = constp.tile([P, 3 * P], f32)
            nc.sync.dma_start(out=w_raw[:, :], in_=wmat[:, :])
            w_sb = constp.tile([P, 3 * P], f32r)
            nc.vector.tensor_copy(w_sb[:, :], w_raw[:, :])

            slots_per_call = rows_per_call // P
            for call in range(n_calls):
                gbuf = gath.tile([P, GPC * 9, B], f32, tag="gbuf")
                for sub in range(3):
                    col0 = call * win_cols + sub * idx_cols
                    nc.gpsimd.dma_gather(
                        out_ap=gbuf[
                            :, sub * slots_per_call : (sub + 1) * slots_per_call, :
                        ],
                        in_ap=geom[:, :],
                        idxs_ap=idx_sb[:, col0 : col0 + idx_cols],
                        num_idxs=rows_per_call,
                        num_idxs_reg=rows_per_call,
                        elem_size=B,
                    )
                for gg in range(GPC):
                    g = call * GPC + gg
                    dotP = psum.tile([P, B], f32, tag="dotP")
                    n1P = psum.tile([P, B], f32, tag="n1P")
                    n2P = psum.tile([P, B], f32, tag="n2P")
                    for j in range(3):
                        ga = gbuf[:, 9 * gg + 3 * j + 0, :]
                        gb = gbuf[:, 9 * gg + 3 * j + 1, :]
                        gc = gbuf[:, 9 * gg + 3 * j + 2, :]
                        d1 = work.tile([P, B], f32, tag="d1")
                        d2 = work.tile([P, B], f32, tag="d2")
                        m = work.tile([P, B], f32r, tag="m")
                        q1 = work.tile([P, B], f32r, tag="q1")
                        q2 = work.tile([P, B], f32r, tag="q2")
                        nc.vector.tensor_sub(d1[:, :], ga, gb)
                        nc.vector.tensor_sub(d2[:, :], gc, gb)
                        nc.gpsimd.tensor_mul(m[:, :], d1[:, :], d2[:, :])
                        nc.scalar.square(q1[:, :], d1[:, :])
                        nc.scalar.square(q2[:, :], d2[:, :])
                        wj = w_sb[:, j * P : (j + 1) * P]
                        st = dict(start=(j == 0), stop=(j == 2))
                        nc.tensor.matmul(
                            out=dotP[:, :], lhsT=wj, rhs=m[:, :], **st
                        )
                        nc.tensor.matmul(
                            out=n1P[:, :], lhsT=wj, rhs=q1[:, :], **st
                        )
                        nc.tensor.matmul(
                            out=n2P[:, :], lhsT=wj, rhs=q2[:, :], **st
                        )

                    s1 = tailp.tile([P, B], f32, tag="s1")
                    s2 = tailp.tile([P, B], f32, tag="s2")
                    s = tailp.tile([P, B], f32, tag="s")
                    r = tailp.tile([P, B], f32, tag="r")
                    res = tailp.tile([P, B], f32, tag="res")
                    nc.scalar.sqrt(s1[:, :], n1P[:, :])
                    nc.scalar.sqrt(s2[:, :], n2P[:, :])
                    nc.vector.tensor_mul(s[:, :], s1[:, :], s2[:, :])
                    nc.vector.reciprocal_approx_fast(r[:, :], s[:, :])
                    nc.vector.tensor_mul(res[:, :], dotP[:, :], r[:, :])
                    nc.sync.dma_start(
                        out=out[g * GROUP : (g + 1) * GROUP, :], in_=res[:GROUP, :]
                    )

    nc.compile()
    return nc


def _build_wmat():
    W = np.zeros((P, 3, P), np.float32)
    a = np.arange(APT)
    for j in range(3):
        for c in range(3):
            W[3 * a + c, j, APT * j + a] = 1.0
    return np.ascontiguousarray(W.reshape(P, 3 * P))


def _prep_core_idxs(angles, core, n_groups):
    """int16 wrapped index array (P, n_calls*idx_cols) for this core."""
    pad_angles = n_groups * GROUP
    ang = angles[core * PER_CORE : core * PER_CORE + pad_angles]
    if ang.shape[0] < pad_angles:
        pad = np.zeros((pad_angles - ang.shape[0], 3), ang.dtype)
        ang = np.concatenate([ang, pad], axis=0)
    A = ang.reshape(n_groups, 3, APT, 3)  # [g, j, a_local, role]
    p = np.arange(P)
    al, c = p // 3, p % 3
    valid = p < GROUP
    idx = np.zeros((n_groups, 3, 3, P), np.int16)
    # [g, j, role, p] = 3*atom + coord
    sel = (3 * A[:, :, al[valid], :] + c[valid][None, None, :, None]).transpose(
        0, 1, 3, 2
    )
    idx[:, :, :, valid] = sel.astype(np.int16)
    n_subcalls = (n_groups // GPC) * 3
    flat = idx.reshape(n_subcalls, GPC * 9 * P // 3)  # rows per dma_gather call
    # wrap: index i lives at partition i%16, free column i//16; replicate x8
    wr = flat.reshape(n_subcalls, -1, 16)  # [call, s, i]
    wrapped = np.tile(wr.transpose(0, 2, 1), (1, 8, 1))  # [call, 128, S]
    return np.ascontiguousarray(
        wrapped.transpose(1, 0, 2).reshape(P, -1)
    )


def kernel(input, angles, _trace=False, _trace_kwargs=None):
    input = np.ascontiguousarray(np.asarray(input, dtype=np.float32))
    angles = np.ascontiguousarray(np.asarray(angles, dtype=np.int32))
    n_atoms = input.shape[0]
    batch = input.shape[2]
    geom = input.reshape(3 * n_atoms, batch)

    key = (n_atoms, N_GROUPS, batch)
    if key not in _NC_CACHE:
        _NC_CACHE[key] = build_nc(*key)
    nc = _NC_CACHE[key]

    wm = _build_wmat()
    in_maps = [
        {"geom": geom, "idxs": _prep_core_idxs(angles, c, N_GROUPS), "wmat": wm}
        for c in range(N_CORES)
    ]
    kw = {}
    if _trace:
        kw["trace"] = True
        kw.update(_trace_kwargs or {})
    res = run_bass_kernel_spmd(nc, in_maps, core_ids=list(range(N_CORES)), **kw)
    outs = [res.results[c]["out"][:PER_CORE] for c in range(N_CORES)]
    full = np.concatenate(outs, axis=0)
    if _trace:
        return full, res
    return full


# revision 13
# speedup vs baseline: 1.1021x; 1.0019x over previous
"""Trainium2 Bass kernel: batched bond-angle cosines (gather + vector math).

Problem: geometry (n_atoms, 3, batch) f32, angle triplets (n_angles, 3) int32.
Output: cos(angle) per (triplet, frame) = (n_angles, batch) f32.

Architecture (v3):
- Shard angles across 8 cores (8192 each, padded to 66 groups x 126).
- Geometry viewed as a (n_atoms*3, batch) row table; row 3a+c = coords of
  atom a, coordinate c. Partition layout packs 42 angles x 3 coords into
  126 of 128 partitions, so the per-coordinate values of an angle's three
  endpoint vectors live in adjacent partitions.
- Gather via gpsimd.dma_gather (batched DGE gather, int16 indices), one call
  per 2 groups (2304 rows of batch*4 bytes).
- Per 42-angle subtile: d1 = a-b (DVE), d2 = c-b (GpSimd), m = d1*d2 (DVE),
  q1 = d1^2, q2 = d2^2 (ACT). The 3-term coordinate sums (dot, |v1|^2,
  |v2|^2) are done by the PE array as float32r matmuls against a constant
  0/1 selection matrix, accumulating 3 subtiles into one 126-row PSUM tile.
- Tail per group: t = n1*n2 (DVE), s = sqrt(t) (ACT), r ~= 1/s (custom DVE
  reciprocal_approx_fast; r(0) = NaN preserving the reference's 0/0 NaN
  semantics), res = dot*r (DVE). Contiguous 126-row DMA to the output.
"""

import numpy as np

import concourse.tile as tile
from concourse import bacc, bass, mybir
from concourse.bass_utils import run_bass_kernel_spmd

P = 128
APT = 42          # angles per subtile (3 coords each -> 126 partitions)
GROUP = 3 * APT   # angles per PSUM group = 126
GPC = 2           # groups per dma_gather call

N_ATOMS = 2048
N_ANGLES = 65536
BATCH = 512
N_CORES = 8
PER_CORE = N_ANGLES // N_CORES          # 8192
N_GROUPS = -(-PER_CORE // GROUP)        # 66
PAD_ANGLES = N_GROUPS * GROUP           # 8316
N_CALLS = N_GROUPS // GPC               # 33

_NC_CACHE = {}


def build_nc(n_atoms=N_ATOMS, n_groups=N_GROUPS, batch=BATCH):
    assert n_groups % GPC == 0
    n_calls = n_groups // GPC
    B = batch
    rows_per_window = GPC * 9 * P  # 2 groups x (3 subtiles x 3 roles) x 128
    rows_per_call = rows_per_window // 3  # dma_gather caps at 1024 idxs/call
    f32 = mybir.dt.float32
    f32r = mybir.dt.float32r
    i16 = mybir.dt.int16

    nc = bacc.Bacc(debug=False, num_swdge_queues=4)

    geom = nc.declare_dram_parameter("geom", [3 * n_atoms, B], f32, isOutput=False)
    idxs = nc.declare_dram_parameter(
        "idxs", [P, n_calls * rows_per_window // 16], i16, isOutput=False
    )
    wmat = nc.declare_dram_parameter("wmat", [P, 3 * P], f32, isOutput=False)
    out = nc.declare_dram_parameter("out", [n_groups * GROUP, B], f32, isOutput=True)

    idx_cols = rows_per_call // 16   # i16 columns per dma_gather call
    win_cols = rows_per_window // 16

    with tile.TileContext(nc) as tc:
        with (
            tc.tile_pool(name="const", bufs=1) as constp,
            tc.tile_pool(name="gath", bufs=3) as gath,
            tc.tile_pool(name="work", bufs=3) as work,
            tc.tile_pool(name="tailp", bufs=2) as tailp,
            tc.tile_pool(name="psum", bufs=2, space="PSUM") as psum,
        ):
            idx_sb = constp.tile([P, n_calls * win_cols], i16)
            nc.sync.dma_start(out=idx_sb[:, :], in_=idxs[:, :])
            w_raw = constp.tile([P, 3 * P], f32)
            nc.sync.dma_start(out=w_raw[:, :], in_=wmat[:, :])
            w_sb = constp.tile([P, 3 * P], f32r)
            nc.vector.tensor_copy(w_sb[:, :], w_raw[:, :])

            slots_per_call = rows_per_call // P
            for call in range(n_calls):
                gbuf = gath.tile([P, GPC * 9, B], f32, tag="gbuf")
                for sub in range(3):
                    col0 = call * win_cols + sub * idx_cols
                    nc.gpsimd.dma_gather(
                        out_ap=gbuf[
                            :, sub * slots_per_call : (sub + 1) * slots_per_call, :
                        ],
                        in_ap=geom[:, :],
                        idxs_ap=idx_sb[:, col0 : col0 + idx_cols],
                        num_idxs=rows_per_call,
                        num_idxs_reg=rows_per_call,
                        elem_size=B,
                        queue_num=(call * 3 + sub) % 4,
                    )
                for gg in range(GPC):
                    g = call * GPC + gg
                    dotP = psum.tile([P, B], f32, tag="dotP")
                    n1P = psum.tile([P, B], f32, tag="n1P")
                    n2P = psum.tile([P, B], f32, tag="n2P")
                    for j in range(3):
                        ga = gbuf[:, 9 * gg + 3 * j + 0, :]
                        gb = gbuf[:, 9 * gg + 3 * j + 1, :]
                        gc = gbuf[:, 9 * gg + 3 * j + 2, :]
                        d1 = work.tile([P, B], f32, tag="d1")
                        d2 = work.tile([P, B], f32, tag="d2")
                        m = work.tile([P, B], f32r, tag="m")
                        q1 = work.tile([P, B], f32r, tag="q1")
                        q2 = work.tile([P, B], f32r, tag="q2")
                        nc.vector.tensor_sub(d1[:, :], ga, gb)
                        nc.vector.tensor_sub(d2[:, :], gc, gb)
                        nc.gpsimd.tensor_mul(m[:, :], d1[:, :], d2[:, :])
                        nc.scalar.square(q1[:, :], d1[:, :])
                        nc.scalar.square(q2[:, :], d2[:, :])
                        wj = w_sb[:, j * P : (j + 1) * P]
                        st = dict(start=(j == 0), stop=(j == 2))
                        nc.tensor.matmul(
                            out=dotP[:, :], lhsT=wj, rhs=m[:, :], **st
                        )
                        nc.tensor.matmul(
                            out=n1P[:, :], lhsT=wj, rhs=q1[:, :], **st
                        )
                        nc.tensor.matmul(
                            out=n2P[:, :], lhsT=wj, rhs=q2[:, :], **st
                        )

                    s1 = tailp.tile([P, B], f32, tag="s1")
                    s2 = tailp.tile([P, B], f32, tag="s2")
                    s = tailp.tile([P, B], f32, tag="s")
                    r = tailp.tile([P, B], f32, tag="r")
                    res = tailp.tile([P, B], f32, tag="res")
                    nc.scalar.sqrt(s1[:, :], n1P[:, :])
                    nc.scalar.sqrt(s2[:, :], n2P[:, :])
                    nc.vector.tensor_mul(s[:, :], s1[:, :], s2[:, :])
                    nc.vector.reciprocal_approx_fast(r[:, :], s[:, :])
                    nc.vector.tensor_mul(res[:, :], dotP[:, :], r[:, :])
                    nc.sync.dma_start(
                        out=out[g * GROUP : (g + 1) * GROUP, :], in_=res[:GROUP, :]
                    )

    nc.compile()
    return nc


def _build_wmat():
    W = np.zeros((P, 3, P), np.float32)
    a = np.arange(APT)
    for j in range(3):
        for c in range(3):
            W[3 * a + c, j, APT * j + a] = 1.0
    return np.ascontiguousarray(W.reshape(P, 3 * P))


def _prep_core_idxs(angles, core, n_groups):
    """int16 wrapped index array (P, n_calls*idx_cols) for this core."""
    pad_angles = n_groups * GROUP
    ang = angles[core * PER_CORE : core * PER_CORE + pad_angles]
    if ang.shape[0] < pad_angles:
        pad = np.zeros((pad_angles - ang.shape[0], 3), ang.dtype)
        ang = np.concatenate([ang, pad], axis=0)
    A = ang.reshape(n_groups, 3, APT, 3)  # [g, j, a_local, role]
    p = np.arange(P)
    al, c = p // 3, p % 3
    valid = p < GROUP
    idx = np.zeros((n_groups, 3, 3, P), np.int16)
    # [g, j, role, p] = 3*atom + coord
    sel = (3 * A[:, :, al[valid], :] + c[valid][None, None, :, None]).transpose(
        0, 1, 3, 2
    )
    idx[:, :, :, valid] = sel.astype(np.int16)
    n_subcalls = (n_groups // GPC) * 3
    flat = idx.reshape(n_subcalls, GPC * 9 * P // 3)  # rows per dma_gather call
    # wrap: index i lives at partition i%16, free column i//16; replicate x8
    wr = flat.reshape(n_subcalls, -1, 16)  # [call, s, i]
    wrapped = np.tile(wr.transpose(0, 2, 1), (1, 8, 1))  # [call, 128, S]
    return np.ascontiguousarray(
        wrapped.transpose(1, 0, 2).reshape(P, -1)
    )


def kernel(input, angles, _trace=False, _trace_kwargs=None):
    input = np.ascontiguousarray(np.asarray(input, dtype=np.float32))
    angles = np.ascontiguousarray(np.asarray(angles, dtype=np.int32))
    n_atoms = input.shape[0]
    batch = input.shape[2]
    geom = input.reshape(3 * n_atoms, batch)

    key = (n_atoms, N_GROUPS, batch)
    if key not in _NC_CACHE:
        _NC_CACHE[key] = build_nc(*key)
    nc = _NC_CACHE[key]

    wm = _build_wmat()
    in_maps = [
        {"geom": geom, "idxs": _prep_core_idxs(angles, c, N_GROUPS), "wmat": wm}
        for c in range(N_CORES)
    ]
    kw = {}
    if _trace:
        kw["trace"] = True
        kw.update(_trace_kwargs or {})
    res = run_bass_kernel_spmd(nc, in_maps, core_ids=list(range(N_CORES)), **kw)
    outs = [res.results[c]["out"][:PER_CORE] for c in range(N_CORES)]
    full = np.concatenate(outs, axis=0)
    if _trace:
        return full, res
    return full


# revision 14
# speedup vs baseline: 3.0764x; 2.7914x over previous
"""Trainium2 Bass kernel: batched bond-angle cosines (gather + vector math).

Problem: geometry (n_atoms, 3, batch) f32, angle triplets (n_angles, 3) int32.
Output: cos(angle) per (triplet, frame) = (n_angles, batch) f32.

Architecture (v4):
- Shard angles across 8 cores (8192 each, 64 tiles of 128 angles).
- Geometry as a (n_atoms, 3*batch) f32 row table (6KB rows). Per tile and
  role, an indirect DGE DMA gathers the 128 endpoint-atom rows into SBUF
  ([x|y|z] planar, 512 frames per coordinate).
- Per tile: d1 = a-b, d2 = c-b, m = d1*d2, q1 = d1^2, q2 = d2^2 as
  1536-wide ops; coordinate sums as 512-wide adds. All flexible 2-input ops
  are emitted engine-agnostic (nc.any) so the Tile scheduler balances them
  across DVE and ACT; gathers live on GpSimd.
- Tail: t = n1*n2, s = sqrt(t) (ACT), r ~= 1/s via the custom-DVE
  reciprocal_approx_fast (r(0) = NaN, preserving the reference's 0/0 NaN
  semantics), res = dot*r. Contiguous 128-row DMA to the output.
"""

import numpy as np

import concourse.tile as tile
from concourse import bacc, bass, mybir
from concourse.bass_utils import run_bass_kernel_spmd

P = 128

N_ATOMS = 2048
N_ANGLES = 65536
BATCH = 512
N_CORES = 8
PER_CORE = N_ANGLES // N_CORES  # 8192
N_TILES = PER_CORE // P  # 64

_NC_CACHE = {}


def build_nc(n_atoms=N_ATOMS, per_core=PER_CORE, batch=BATCH):
    n_tiles = per_core // P
    B = batch
    f32 = mybir.dt.float32
    i32 = mybir.dt.int32

    nc = bacc.Bacc(debug=False)

    geom = nc.declare_dram_parameter("geom", [n_atoms, 3 * B], f32, isOutput=False)
    # idxs[p, t*3 + r] = angles[t*128 + p, r]
    idxs = nc.declare_dram_parameter("idxs", [P, 3 * n_tiles], i32, isOutput=False)
    out = nc.declare_dram_parameter("out", [per_core, B], f32, isOutput=True)

    with tile.TileContext(nc) as tc:
        with (
            tc.tile_pool(name="idxp", bufs=1) as idxp,
            tc.tile_pool(name="gath", bufs=3) as gath,
            tc.tile_pool(name="work", bufs=2) as work,
            tc.tile_pool(name="outp", bufs=3) as outp,
        ):
            idx_sb = idxp.tile([P, 3 * n_tiles], i32)
            nc.sync.dma_start(out=idx_sb[:, :], in_=idxs[:, :])

            for t in range(n_tiles):
                ga = gath.tile([P, 3 * B], f32, tag="ga")
                gb = gath.tile([P, 3 * B], f32, tag="gb")
                gc = gath.tile([P, 3 * B], f32, tag="gc")
                for role, g in enumerate((ga, gb, gc)):
                    nc.gpsimd.indirect_dma_start(
                        out=g[:, :],
                        out_offset=None,
                        in_=geom[:, :],
                        in_offset=bass.IndirectOffsetOnAxis(
                            ap=idx_sb[:, 3 * t + role : 3 * t + role + 1],
                            axis=0,
                        ),
                    )

                d1 = work.tile([P, 3 * B], f32, tag="d1")
                d2 = work.tile([P, 3 * B], f32, tag="d2")
                m = work.tile([P, 3 * B], f32, tag="m")
                q1 = work.tile([P, 3 * B], f32, tag="q1")
                q2 = work.tile([P, 3 * B], f32, tag="q2")

                nc.any.tensor_sub(d1[:, :], ga[:, :], gb[:, :])
                nc.any.tensor_sub(d2[:, :], gc[:, :], gb[:, :])
                nc.any.tensor_mul(m[:, :], d1[:, :], d2[:, :])
                nc.any.tensor_mul(q1[:, :], d1[:, :], d1[:, :])
                nc.any.tensor_mul(q2[:, :], d2[:, :], d2[:, :])

                dot = work.tile([P, B], f32, tag="dot")
                n1 = work.tile([P, B], f32, tag="n1")
                n2 = work.tile([P, B], f32, tag="n2")
                t_ = work.tile([P, B], f32, tag="t_")
                s = work.tile([P, B], f32, tag="s")
                r = work.tile([P, B], f32, tag="r")

                mx, my, mz = (m[:, i * B : (i + 1) * B] for i in range(3))
                ax, ay, az = (q1[:, i * B : (i + 1) * B] for i in range(3))
                bx, by, bz = (q2[:, i * B : (i + 1) * B] for i in range(3))
                nc.any.tensor_add(dot[:, :], mx, my)
                nc.any.tensor_add(dot[:, :], dot[:, :], mz)
                nc.any.tensor_add(n1[:, :], ax, ay)
                nc.any.tensor_add(n1[:, :], n1[:, :], az)
                nc.any.tensor_add(n2[:, :], bx, by)
                nc.any.tensor_add(n2[:, :], n2[:, :], bz)

                nc.any.tensor_mul(t_[:, :], n1[:, :], n2[:, :])
                nc.scalar.sqrt(s[:, :], t_[:, :])
                nc.vector.reciprocal_approx_fast(r[:, :], s[:, :])

                res = outp.tile([P, B], f32, tag="res")
                nc.any.tensor_mul(res[:, :], dot[:, :], r[:, :])
                nc.sync.dma_start(
                    out=out[t * P : (t + 1) * P, :], in_=res[:, :]
                )

    nc.compile()
    return nc


def _prep_core_inputs(geom2d, angles, core):
    ang = angles[core * PER_CORE : (core + 1) * PER_CORE]
    idxs = np.ascontiguousarray(
        ang.reshape(N_TILES, P, 3).transpose(1, 0, 2).reshape(P, 3 * N_TILES)
    )
    return {"geom": geom2d, "idxs": idxs}


def kernel(input, angles, _trace=False, _trace_kwargs=None):
    input = np.ascontiguousarray(np.asarray(input, dtype=np.float32))
    angles = np.ascontiguousarray(np.asarray(angles, dtype=np.int32))
    assert input.shape == (N_ATOMS, 3, BATCH)
    assert angles.shape == (N_ANGLES, 3)

    geom2d = input.reshape(N_ATOMS, 3 * BATCH)

    key = (N_ATOMS, PER_CORE, BATCH)
    if key not in _NC_CACHE:
        _NC_CACHE[key] = build_nc(*key)
    nc = _NC_CACHE[key]

    in_maps = [_prep_core_inputs(geom2d, angles, c) for c in range(N_CORES)]
    kw = {}
    if _trace:
        kw["trace"] = True
        kw.update(_trace_kwargs or {})
    res = run_bass_kernel_spmd(nc, in_maps, core_ids=list(range(N_CORES)), **kw)
    outs = [res.results[c]["out"] for c in range(N_CORES)]
    full = np.concatenate(outs, axis=0)
    if _trace:
        return full, res
    return full


# revision 15
# speedup vs baseline: 4.2706x; 1.3882x over previous
"""Trainium2 Bass kernel: batched bond-angle cosines (gather + vector math).

Problem: geometry (n_atoms, 3, batch) f32, angle triplets (n_angles, 3) int32.
Output: cos(angle) per (triplet, frame) = (n_angles, batch) f32.

Architecture (v4):
- Shard angles across 8 cores (8192 each, 64 tiles of 128 angles).
- Geometry as a (n_atoms, 3*batch) f32 row table (6KB rows). Per tile and
  role, an indirect DGE DMA gathers the 128 endpoint-atom rows into SBUF
  ([x|y|z] planar, 512 frames per coordinate).
- Per tile: d1 = a-b, d2 = c-b on DVE (f32 in, fp16 out: rounding the
  difference keeps the error relative to |v|). Products m = d1*d2 and the
  coordinate-sum adds run in fp16 on DVE (2x perf mode); squares q = d^2 on
  the Scalar engine. The host pre-scales the geometry by 8 so fp16 squares
  of the smallest |v| stay in the normal range while sums stay < 65504
  (cos is scale-invariant). Gathers live on GpSimd.
- Tail: t = n1*n2, s = sqrt(t) (ACT), r ~= 1/s via the custom-DVE
  reciprocal_approx_fast (r(0) = NaN, preserving the reference's 0/0 NaN
  semantics), res = dot*r. Contiguous 128-row DMA to the output.
"""

import numpy as np

import concourse.tile as tile
from concourse import bacc, bass, mybir
from concourse.bass_utils import run_bass_kernel_spmd

P = 128

N_ATOMS = 2048
N_ANGLES = 65536
BATCH = 512
N_CORES = 8
PER_CORE = N_ANGLES // N_CORES  # 8192
N_TILES = PER_CORE // P  # 64

_NC_CACHE = {}


def build_nc(n_atoms=N_ATOMS, per_core=PER_CORE, batch=BATCH):
    n_tiles = per_core // P
    B = batch
    f32 = mybir.dt.float32
    f16 = mybir.dt.float16
    i32 = mybir.dt.int32

    nc = bacc.Bacc(debug=False)

    geom = nc.declare_dram_parameter("geom", [n_atoms, 3 * B], f32, isOutput=False)
    # idxs[p, t*3 + r] = angles[t*128 + p, r]
    idxs = nc.declare_dram_parameter("idxs", [P, 3 * n_tiles], i32, isOutput=False)
    out = nc.declare_dram_parameter("out", [per_core, B], f32, isOutput=True)

    with tile.TileContext(nc) as tc:
        with (
            tc.tile_pool(name="idxp", bufs=1) as idxp,
            tc.tile_pool(name="gath", bufs=3) as gath,
            tc.tile_pool(name="work", bufs=2) as work,
            tc.tile_pool(name="outp", bufs=3) as outp,
        ):
            idx_sb = idxp.tile([P, 3 * n_tiles], i32)
            nc.sync.dma_start(out=idx_sb[:, :], in_=idxs[:, :])

            for t in range(n_tiles):
                ga = gath.tile([P, 3 * B], f32, tag="ga")
                gb = gath.tile([P, 3 * B], f32, tag="gb")
                gc = gath.tile([P, 3 * B], f32, tag="gc")
                for role, g in enumerate((ga, gb, gc)):
                    nc.gpsimd.indirect_dma_start(
                        out=g[:, :],
                        out_offset=None,
                        in_=geom[:, :],
                        in_offset=bass.IndirectOffsetOnAxis(
                            ap=idx_sb[:, 3 * t + role : 3 * t + role + 1],
                            axis=0,
                        ),
                    )

                d1 = work.tile([P, 3 * B], f16, tag="d1")
                d2 = work.tile([P, 3 * B], f16, tag="d2")
                m = work.tile([P, 3 * B], f16, tag="m")
                q1 = work.tile([P, 3 * B], f16, tag="q1")
                q2 = work.tile([P, 3 * B], f16, tag="q2")

                nc.vector.tensor_sub(d1[:, :], ga[:, :], gb[:, :])
                nc.vector.tensor_sub(d2[:, :], gc[:, :], gb[:, :])
                nc.vector.tensor_mul(m[:, :], d1[:, :], d2[:, :])
                nc.scalar.square(q1[:, :], d1[:, :])
                nc.scalar.square(q2[:, :], d2[:, :])

                dot = work.tile([P, B], f16, tag="dot")
                n1 = work.tile([P, B], f16, tag="n1")
                n2 = work.tile([P, B], f16, tag="n2")
                t_ = work.tile([P, B], f32, tag="t_")
                s = work.tile([P, B], f32, tag="s")
                r = work.tile([P, B], f32, tag="r")

                mx, my, mz = (m[:, i * B : (i + 1) * B] for i in range(3))
                ax, ay, az = (q1[:, i * B : (i + 1) * B] for i in range(3))
                bx, by, bz = (q2[:, i * B : (i + 1) * B] for i in range(3))
                nc.vector.tensor_add(dot[:, :], mx, my)
                nc.vector.tensor_add(dot[:, :], dot[:, :], mz)
                nc.vector.tensor_add(n1[:, :], ax, ay)
                nc.vector.tensor_add(n1[:, :], n1[:, :], az)
                nc.gpsimd.tensor_add(n2[:, :], bx, by)
                nc.gpsimd.tensor_add(n2[:, :], n2[:, :], bz)

                nc.vector.tensor_mul(t_[:, :], n1[:, :], n2[:, :])
                nc.scalar.sqrt(s[:, :], t_[:, :])
                nc.vector.reciprocal_approx_fast(r[:, :], s[:, :])

                res = outp.tile([P, B], f32, tag="res")
                nc.vector.tensor_mul(res[:, :], dot[:, :], r[:, :])
                nc.sync.dma_start(
                    out=out[t * P : (t + 1) * P, :], in_=res[:, :]
                )

    nc.compile()
    return nc


def _prep_core_inputs(geom2d, angles, core):
    ang = angles[core * PER_CORE : (core + 1) * PER_CORE]
    idxs = np.ascontiguousarray(
        ang.reshape(N_TILES, P, 3).transpose(1, 0, 2).reshape(P, 3 * N_TILES)
    )
    return {"geom": geom2d, "idxs": idxs}


def kernel(input, angles, _trace=False, _trace_kwargs=None):
    input = np.ascontiguousarray(np.asarray(input, dtype=np.float32))
    angles = np.ascontiguousarray(np.asarray(angles, dtype=np.int32))
    assert input.shape == (N_ATOMS, 3, BATCH)
    assert angles.shape == (N_ANGLES, 3)

    # scale by 8 so fp16 squares of the smallest nonzero |v| stay normal
    # while |v1|^2 sums stay below fp16 max; cos() is scale-invariant.
    geom2d = (input.reshape(N_ATOMS, 3 * BATCH) * 8.0).astype(np.float32)

    key = (N_ATOMS, PER_CORE, BATCH)
    if key not in _NC_CACHE:
        _NC_CACHE[key] = build_nc(*key)
    nc = _NC_CACHE[key]

    in_maps = [_prep_core_inputs(geom2d, angles, c) for c in range(N_CORES)]
    kw = {}
    if _trace:
        kw["trace"] = True
        kw.update(_trace_kwargs or {})
    res = run_bass_kernel_spmd(nc, in_maps, core_ids=list(range(N_CORES)), **kw)
    outs = [res.results[c]["out"] for c in range(N_CORES)]
    full = np.concatenate(outs, axis=0)
    if _trace:
        return full, res
    return full


# revision 16
# speedup vs baseline: 4.5639x; 1.0687x over previous
"""Trainium2 Bass kernel: batched bond-angle cosines (gather + vector math).

Problem: geometry (n_atoms, 3, batch) f32, angle triplets (n_angles, 3) int32.
Output: cos(angle) per (triplet, frame) = (n_angles, batch) f32.

Architecture (v4):
- Shard angles across 8 cores (8192 each, 64 tiles of 128 angles).
- Geometry as a (n_atoms, 3*batch) f32 row table (6KB rows). Per tile and
  role, an indirect DGE DMA gathers the 128 endpoint-atom rows into SBUF
  ([x|y|z] planar, 512 frames per coordinate).
- Per tile: d1 = a-b, d2 = c-b on DVE (f32 in, fp16 out: rounding the
  difference keeps the error relative to |v|). Products m = d1*d2 and the
  coordinate-sum adds run in fp16 on DVE (2x perf mode); squares q = d^2 on
  the Scalar engine. The host pre-scales the geometry by 8 so fp16 squares
  of the smallest |v| stay in the normal range while sums stay < 65504
  (cos is scale-invariant). Gathers live on GpSimd.
- Tail: t = n1*n2, s = sqrt(t) (ACT), r ~= 1/s via the custom-DVE
  reciprocal_approx_fast (r(0) = NaN, preserving the reference's 0/0 NaN
  semantics), res = dot*r. Contiguous 128-row DMA to the output.
"""

import numpy as np

import concourse.tile as tile
from concourse import bacc, bass, mybir
from concourse.bass_utils import run_bass_kernel_spmd

P = 128

N_ATOMS = 2048
N_ANGLES = 65536
BATCH = 512
N_CORES = 8
PER_CORE = N_ANGLES // N_CORES  # 8192
N_TILES = PER_CORE // P  # 64

_NC_CACHE = {}


def build_nc(n_atoms=N_ATOMS, per_core=PER_CORE, batch=BATCH):
    n_tiles = per_core // P
    B = batch
    f32 = mybir.dt.float32
    f16 = mybir.dt.float16
    i32 = mybir.dt.int32

    nc = bacc.Bacc(debug=False)

    geom = nc.declare_dram_parameter("geom", [n_atoms, 3 * B], f32, isOutput=False)
    # idxs[p, t*3 + r] = angles[t*128 + p, r]
    idxs = nc.declare_dram_parameter("idxs", [P, 3 * n_tiles], i32, isOutput=False)
    out = nc.declare_dram_parameter("out", [per_core, B], f32, isOutput=True)

    with tile.TileContext(nc) as tc:
        with (
            tc.tile_pool(name="idxp", bufs=1) as idxp,
            tc.tile_pool(name="gath", bufs=3) as gath,
            tc.tile_pool(name="work", bufs=2) as work,
            tc.tile_pool(name="outp", bufs=3) as outp,
        ):
            idx_sb = idxp.tile([P, 3 * n_tiles], i32)
            nc.sync.dma_start(out=idx_sb[:, :], in_=idxs[:, :])

            for t in range(n_tiles):
                ga = gath.tile([P, 3 * B], f32, tag="ga")
                gb = gath.tile([P, 3 * B], f32, tag="gb")
                gc = gath.tile([P, 3 * B], f32, tag="gc")
                for role, g in enumerate((ga, gb, gc)):
                    nc.gpsimd.indirect_dma_start(
                        out=g[:, :],
                        out_offset=None,
                        in_=geom[:, :],
                        in_offset=bass.IndirectOffsetOnAxis(
                            ap=idx_sb[:, 3 * t + role : 3 * t + role + 1],
                            axis=0,
                        ),
                    )

                d1 = work.tile([P, 3 * B], f16, tag="d1")
                d2 = work.tile([P, 3 * B], f16, tag="d2")
                # pk = [m | q1 | q2] packed so the three coordinate-sum
                # trees collapse into two wide (128,3,512) adds
                pk = work.tile([P, 3, 3 * B], f16, tag="pk")

                nc.vector.tensor_sub(d1[:, :], ga[:, :], gb[:, :])
                nc.vector.tensor_sub(d2[:, :], gc[:, :], gb[:, :])
                nc.vector.tensor_mul(pk[:, 0, :], d1[:, :], d2[:, :])
                nc.scalar.square(pk[:, 1, :], d1[:, :])
                nc.scalar.square(pk[:, 2, :], d2[:, :])

                # su = [dot | n1 | n2]
                su = work.tile([P, 3, B], f16, tag="su")
                t_ = work.tile([P, B], f32, tag="t_")
                s = work.tile([P, B], f32, tag="s")
                r = work.tile([P, B], f32, tag="r")

                nc.vector.tensor_add(
                    su[:, :, :], pk[:, :, 0:B], pk[:, :, B : 2 * B]
                )
                nc.vector.tensor_add(
                    su[:, :, :], su[:, :, :], pk[:, :, 2 * B : 3 * B]
                )
                dot, n1, n2 = (su[:, i, :] for i in range(3))

                nc.gpsimd.tensor_mul(t_[:, :], n1, n2)
                nc.scalar.sqrt(s[:, :], t_[:, :])
                nc.vector.reciprocal_approx_fast(r[:, :], s[:, :])

                res = outp.tile([P, B], f32, tag="res")
                nc.vector.tensor_mul(res[:, :], dot, r[:, :])
                nc.sync.dma_start(
                    out=out[t * P : (t + 1) * P, :], in_=res[:, :]
                )

    nc.compile()
    return nc


def _prep_core_inputs(geom2d, angles, core):
    ang = angles[core * PER_CORE : (core + 1) * PER_CORE]
    idxs = np.ascontiguousarray(
        ang.reshape(N_TILES, P, 3).transpose(1, 0, 2).reshape(P, 3 * N_TILES)
    )
    return {"geom": geom2d, "idxs": idxs}


def kernel(input, angles, _trace=False, _trace_kwargs=None):
    input = np.ascontiguousarray(np.asarray(input, dtype=np.float32))
    angles = np.ascontiguousarray(np.asarray(angles, dtype=np.int32))
    assert input.shape == (N_ATOMS, 3, BATCH)
    assert angles.shape == (N_ANGLES, 3)

    # scale by 8 so fp16 squares of the smallest nonzero |v| stay normal
    # while |v1|^2 sums stay below fp16 max; cos() is scale-invariant.
    geom2d = (input.reshape(N_ATOMS, 3 * BATCH) * 8.0).astype(np.float32)

    key = (N_ATOMS, PER_CORE, BATCH)
    if key not in _NC_CACHE:
        _NC_CACHE[key] = build_nc(*key)
    nc = _NC_CACHE[key]

    in_maps = [_prep_core_inputs(geom2d, angles, c) for c in range(N_CORES)]
    kw = {}
    if _trace:
        kw["trace"] = True
        kw.update(_trace_kwargs or {})
    res = run_bass_kernel_spmd(nc, in_maps, core_ids=list(range(N_CORES)), **kw)
    outs = [res.results[c]["out"] for c in range(N_CORES)]
    full = np.concatenate(outs, axis=0)
    if _trace:
        return full, res
    return full
